# revision 1
# baseline (speedup 1.0000x reference)
import sys
sys.path.insert(0, '/opt/trn_rl_repo')
import numpy as np
import concourse.bass as bass
import concourse.tile as tile
from concourse import bacc, mybir
from concourse.bass_utils import run_bass_kernel_spmd
from concourse.masks import make_identity

F32 = mybir.dt.float32
F32R = mybir.dt.float32r
AF = mybir.ActivationFunctionType
ALU = mybir.AluOpType

B, NOBJ, DB, DO, DG, DSEM = 131072, 5, 10, 15, 35, 7
HID, DPHI, DA = 256, 256, 4
NCORES = 8
RPC = B // NCORES            # rows per core
RMT = 512                    # rows per macro tile
LOG_SIG_MIN, LOG_SIG_MAX = -20.0, 2.0

_CACHE = {}
LAST_EXEC_NS = None


def _build(n_mt, repeat=1):
    nc = bacc.Bacc("TRN2", target_bir_lowering=False, debug=False)
    rpc = n_mt * RMT
    obs_d = nc.declare_dram_parameter("obs", [rpc, 85], F32, isOutput=False)
    gp_d = nc.declare_dram_parameter("gp", [rpc, 35], F32, isOutput=False)
    agp_d = nc.declare_dram_parameter("agp", [rpc, 35], F32, isOutput=False)
    w1row_d = nc.declare_dram_parameter("w1row", [128, 256], F32, isOutput=False)
    w2_d = nc.declare_dram_parameter("w2", [128, 512], F32, isOutput=False)
    rv_d = nc.declare_dram_parameter("rv", [128, 512], F32, isOutput=False)
    mw_d = nc.declare_dram_parameter("mw", [128, 16], F32, isOutput=False)
    qkw_d = nc.declare_dram_parameter("qkw", [128, 4], F32, isOutput=False)
    selqk_d = nc.declare_dram_parameter("selqk", [2, 250], F32, isOutput=False)
    qkb2_d = nc.declare_dram_parameter("qkb2", [2, 1], F32, isOutput=False)
    sumd_d = nc.declare_dram_parameter("sumd", [25, 5], F32, isOutput=False)
    seld_d = nc.declare_dram_parameter("seld", [5, 25], F32, isOutput=False)
    sumw_d = nc.declare_dram_parameter("sumw", [25, 5], F32, isOutput=False)
    b1_d = nc.declare_dram_parameter("b1", [128, 2], F32, isOutput=False)
    b2_d = nc.declare_dram_parameter("b2", [128, 2], F32, isOutput=False)
    rvb_d = nc.declare_dram_parameter("rvb", [128, 2], F32, isOutput=False)
    mb_d = nc.declare_dram_parameter("mb", [8, 1], F32, isOutput=False)
    em5_d = nc.declare_dram_parameter("em5", [5, 640], F32, isOutput=False)
    out_d = nc.declare_dram_parameter("out", [rpc, 8], F32, isOutput=True)

    with tile.TileContext(nc) as tc, \
         nc.allow_low_precision(reason="float32r matmul inputs are explicitly rounded"):
        with tc.tile_pool(name="singles", bufs=1) as singles, \
             tc.tile_pool(name="stage", bufs=1) as stage, \
             tc.tile_pool(name="rm", bufs=4) as rmp, \
             tc.tile_pool(name="gag", bufs=4) as gag, \
             tc.tile_pool(name="xt", bufs=3) as xtp, \
             tc.tile_pool(name="h1", bufs=6) as h1p, \
             tc.tile_pool(name="h2", bufs=22) as h2p, \
             tc.tile_pool(name="tmp", bufs=6) as tmpp, \
             tc.tile_pool(name="attn_sb", bufs=3) as asb, \
             tc.tile_pool(name="rsb", bufs=4) as rsbp, \
             tc.tile_pool(name="osb", bufs=4) as osbp, \
             tc.tile_pool(name="tp", bufs=1, space="PSUM") as tpp, \
             tc.tile_pool(name="mm", bufs=4, space="PSUM") as mmp, \
             tc.tile_pool(name="attn_ps", bufs=3, space="PSUM") as aps:

            # ---- init: identity, weight load + round to f32r ----
            id128 = singles.tile([128, 128], F32)
            make_identity(nc, id128[:])

            def load_round(dram, shape, name):
                st = stage.tile(list(shape), F32, tag=f"st_{name}", name=f"st_{name}")
                nc.sync.dma_start(out=st[:], in_=dram[:])
                rt = singles.tile(list(shape), F32R, tag=f"w_{name}", name=f"w_{name}")
                nc.vector.tensor_copy(rt[:], st[:])
                return rt

            w1row = load_round(w1row_d, (128, 256), "w1")
            w2 = load_round(w2_d, (128, 512), "w2")
            rv = load_round(rv_d, (128, 512), "rv")
            mw = load_round(mw_d, (128, 16), "mw")
            qkw = load_round(qkw_d, (128, 4), "qkw")
            selqk = load_round(selqk_d, (2, 250), "selqk")
            qkb2 = singles.tile([2, 1], F32, name="qkb2")
            nc.sync.dma_start(out=qkb2[:], in_=qkb2_d[:])
            sumd = load_round(sumd_d, (25, 5), "sumd")
            seld = load_round(seld_d, (5, 25), "seld")
            sumw = load_round(sumw_d, (25, 5), "sumw")
            em5 = load_round(em5_d, (5, 640), "em5")
            b1 = singles.tile([128, 2], F32)
            nc.sync.dma_start(out=b1[:], in_=b1_d[:])
            b2 = singles.tile([128, 2], F32)
            nc.sync.dma_start(out=b2[:], in_=b2_d[:])
            rvb = singles.tile([128, 2], F32)
            nc.sync.dma_start(out=rvb[:], in_=rvb_d[:])
            mb = singles.tile([8, 1], F32)
            nc.sync.dma_start(out=mb[:], in_=mb_d[:])

            for t in range(n_mt * repeat):
                row0 = (t % n_mt) * RMT
                # ---- stage 1: merged input DMAs + transposes ----
                gt4 = gag.tile([128, 4, 35], F32, tag="gt", name="gt")
                agt4 = gag.tile([128, 4, 35], F32, tag="agt", name="agt")
                nc.sync.dma_start(out=gt4[:], in_=bass.AP(
                    tensor=gp_d[:, :].tensor, offset=row0 * 35,
                    ap=[[35, 128], [35 * 128, 4], [1, 35]]))
                nc.sync.dma_start(out=agt4[:], in_=bass.AP(
                    tensor=agp_d[:, :].tensor, offset=row0 * 35,
                    ap=[[35, 128], [35 * 128, 4], [1, 35]]))
                rma4 = rmp.tile([128, 4, 128], F32, tag="rma", name="rma")
                rv4 = rma4[:].rearrange("p c (n f) -> p c n f", f=32)
                for n4 in range(4):
                    nc.sync.dma_start(out=rv4[:, :, n4, 0:10], in_=bass.AP(
                        tensor=obs_d[:, :].tensor, offset=row0 * 85,
                        ap=[[85, 128], [85 * 128, 4], [1, 10]]))
                for c4 in range(4):
                    nc.sync.dma_start(out=rv4[:, c4, :, 10:25], in_=bass.AP(
                        tensor=obs_d[:, :].tensor,
                        offset=(row0 + 128 * c4) * 85 + 10,
                        ap=[[85, 128], [15, 4], [1, 15]]))
                g4 = gt4[:].rearrange("p c (n f) -> p c n f", f=7)
                a4 = agt4[:].rearrange("p c (n f) -> p c n f", f=7)
                nc.vector.tensor_sub(rv4[:, :, :, 25:32], g4[:, :, 0:4, :],
                                     a4[:, :, 0:4, :])
                rmb4 = rmp.tile([128, 4, 32], F32, tag="rmb", name="rmb")
                nc.sync.dma_start(out=rmb4[:, :, 0:10], in_=bass.AP(
                    tensor=obs_d[:, :].tensor, offset=row0 * 85,
                    ap=[[85, 128], [85 * 128, 4], [1, 10]]))
                nc.sync.dma_start(out=rmb4[:, :, 10:25], in_=bass.AP(
                    tensor=obs_d[:, :].tensor, offset=row0 * 85 + 70,
                    ap=[[85, 128], [85 * 128, 4], [1, 15]]))
                nc.vector.tensor_sub(rmb4[:, :, 25:32], gt4[:, :, 28:35],
                                     agt4[:, :, 28:35])
                xta = xtp.tile([128, RMT], F32R, tag="xta", name="xta")
                xtb = xtp.tile([32, RMT], F32R, tag="xtb", name="xtb")
                tpa4 = tpp.tile([128, RMT], F32, tag="tp", name="tp")
                tpb4 = aps.tile([32, RMT], F32, tag="attn", name="tpb")
                for j in range(4):
                    nc.tensor.transpose(tpa4[:, j * 128:(j + 1) * 128],
                                        rma4[:, j, :], id128[:])
                    nc.tensor.transpose(tpb4[0:32, j * 128:(j + 1) * 128],
                                        rmb4[:, j, :], id128[:])
                nc.scalar.activation(xta[:], tpa4[:], AF.Copy)
                nc.scalar.activation(xtb[:], tpb4[:], AF.Copy)

                # ---- stage 2+3+4: per-object L1, L2, qk ----
                h2sb = [[None, None] for _ in range(NOBJ)]
                qk2s = [None] * NOBJ
                for n in range(NOBJ):
                    h1sb = [None, None]
                    for h in range(2):
                        ph1 = mmp.tile([128, RMT], F32, tag="mm", name="mm")
                        if n < 4:
                            nc.tensor.matmul(
                                ph1[:], w1row[32 * n:32 * n + 32, 128 * h:128 * h + 128],
                                xta[32 * n:32 * n + 32, :],
                                start=True, stop=True, tile_position=(32 * n, 0))
                        else:
                            nc.tensor.matmul(
                                ph1[:], w1row[0:32, 128 * h:128 * h + 128],
                                xtb[:, :], start=True, stop=True,
                                tile_position=(0, 0))
                        h1sb[h] = h1p.tile([128, RMT], F32R, tag="h1sb", name="h1sb")
                        nc.scalar.activation(h1sb[h][:], ph1[:], AF.Relu,
                                             bias=b1[:, h:h + 1])
                    for h in range(2):
                        pl2 = mmp.tile([128, RMT], F32, tag="mm", name="mm")
                        for k in range(2):
                            nc.tensor.matmul(
                                pl2[:], w2[:, 256 * k + 128 * h:256 * k + 128 * h + 128],
                                h1sb[k][:], start=(k == 0), stop=(k == 1))
                        h2sb[n][h] = h2p.tile([128, RMT], F32R, tag="h2sb", name="h2sb")
                        nc.scalar.activation(h2sb[n][h][:], pl2[:], AF.Relu,
                                             bias=b2[:, h:h + 1])
                    pqk = aps.tile([2, RMT], F32, tag="attn", name="attn")
                    for k in range(2):
                        nc.tensor.matmul(pqk[:], qkw[:, 2 * k:2 * k + 2],
                                         h2sb[n][k][:], start=(k == 0), stop=(k == 1))
                    qk2s[n] = asb.tile([2, RMT], F32R, tag="qk2", name="qk2", bufs=7)
                    nc.vector.tensor_scalar(out=qk2s[n][:], in0=pqk[:],
                                            scalar1=qkb2[:], scalar2=None,
                                            op0=ALU.add)

                # ---- stage 5: attention on 25 partitions ----
                pq25 = aps.tile([25, RMT], F32, tag="attn", name="attn")
                pk25 = aps.tile([25, RMT], F32, tag="attn", name="attn")
                for n in range(NOBJ):
                    nc.tensor.matmul(pq25[:], selqk[:, 25 * n:25 * n + 25],
                                     qk2s[n][:], start=(n == 0), stop=(n == 4))
                for n in range(NOBJ):
                    nc.tensor.matmul(pk25[:], selqk[:, 125 + 25 * n:150 + 25 * n],
                                     qk2s[n][:], start=(n == 0), stop=(n == 4))
                k25 = asb.tile([25, RMT], F32R, tag="k25", name="k25")
                nc.vector.tensor_copy(k25[:], pk25[:])
                s25 = asb.tile([25, RMT], F32, tag="s25", name="s25")
                nc.vector.tensor_mul(s25[:], pq25[:], k25[:])
                e25 = asb.tile([25, RMT], F32R, tag="e25", name="e25")
                nc.scalar.activation(e25[:], s25[:], AF.Exp)
                pden = aps.tile([5, RMT], F32, tag="attn", name="attn")
                nc.tensor.matmul(pden[:], sumd[:], e25[:], start=True, stop=True)
                invden = asb.tile([5, RMT], F32R, tag="invden", name="invden")
                nc.vector.reciprocal(invden[:], pden[:])
                piden = aps.tile([25, RMT], F32, tag="attn", name="attn")
                nc.tensor.matmul(piden[:], seld[:], invden[:], start=True, stop=True)
                a25 = asb.tile([25, RMT], F32R, tag="a25", name="a25")
                nc.vector.tensor_mul(a25[:], e25[:], piden[:])
                pw = aps.tile([5, RMT], F32, tag="attn", name="attn")
                nc.tensor.matmul(pw[:], sumw[:], a25[:], start=True, stop=True)
                wsb = asb.tile([5, RMT], F32R, tag="wsb", name="wsb")
                nc.vector.tensor_copy(wsb[:], pw[:])

                # ---- stage 6: u = sum_m w_m * h2_m, then rho via RV ----
                u = [None, None]
                for m in range(NOBJ):
                    pw128 = aps.tile([128, RMT], F32, tag="attn", name="w128")
                    nc.tensor.matmul(pw128[:], em5[:, 128 * m:128 * m + 128],
                                     wsb[:], start=True, stop=True)
                    for k in range(2):
                        if m == 0:
                            u[k] = tmpp.tile([128, RMT], F32R, tag="u", name="u")
                            nc.vector.tensor_mul(u[k][:], h2sb[0][k][:], pw128[:])
                        else:
                            tmp2 = tmpp.tile([128, RMT], F32R, tag="tmp2", name="tmp2")
                            nc.vector.tensor_mul(tmp2[:], h2sb[m][k][:], pw128[:])
                            nc.vector.tensor_add(u[k][:], u[k][:], tmp2[:])
                pr = [None, None]
                for h in range(2):
                    pr[h] = mmp.tile([128, RMT], F32, tag="mm", name="mm")
                    for k in range(2):
                        nc.tensor.matmul(
                            pr[h][:], rv[:, 256 * k + 128 * h:256 * k + 128 * h + 128],
                            u[k][:], start=(k == 0), stop=(k == 1))
                rsb = [None, None]
                for h in range(2):
                    rsb[h] = rsbp.tile([128, RMT], F32R, tag="rsb", name="rsb")
                    nc.scalar.activation(rsb[h][:], pr[h][:], AF.Relu,
                                         bias=rvb[:, h:h + 1])

                # ---- stage 7: mean/logstd + output transpose ----
                pml = aps.tile([8, RMT], F32, tag="attn", name="attn")
                for k in range(2):
                    nc.tensor.matmul(pml[:], mw[:, 8 * k:8 * k + 8], rsb[k][:],
                                     start=(k == 0), stop=(k == 1))
                mlsb = asb.tile([8, RMT], F32, tag="mlsb", name="mlsb")
                nc.vector.tensor_scalar(out=mlsb[:], in0=pml[:], scalar1=mb[:],
                                        scalar2=None, op0=ALU.add)
                pot4 = aps.tile([128, 32], F32, tag="attn", name="tpo")
                for j in range(4):
                    nc.tensor.transpose(pot4[0:128, j * 8:j * 8 + 8],
                                        mlsb[:, j * 128:(j + 1) * 128],
                                        id128[0:8, 0:8])
                outsb4 = osbp.tile([128, 4, 8], F32, tag="outsb", name="outsb")
                pot4v = pot4[0:128, 0:32].rearrange("p (c f) -> p c f", f=8)
                nc.vector.tensor_copy(outsb4[:, :, 0:4], pot4v[:, :, 0:4])
                nc.vector.tensor_scalar(out=outsb4[:, :, 4:8], in0=pot4v[:, :, 4:8],
                                        scalar1=LOG_SIG_MIN, scalar2=LOG_SIG_MAX,
                                        op0=ALU.max, op1=ALU.min)
                nc.sync.dma_start(out=bass.AP(
                    tensor=out_d[:, :].tensor, offset=row0 * 8,
                    ap=[[8, 128], [8 * 128, 4], [1, 8]]), in_=outsb4[:])
    nc.compile()
    return nc


def _prep_weights(inp):
    f = np.float32
    phi_w1 = np.asarray(inp["phi_w1"], f)
    phi_b1 = np.asarray(inp["phi_b1"], f)
    phi_w2 = np.asarray(inp["phi_w2"], f)
    phi_b2 = np.asarray(inp["phi_b2"], f)
    q_w = np.asarray(inp["q_w"], f); q_b = np.asarray(inp["q_b"], f)
    k_w = np.asarray(inp["k_w"], f); k_b = np.asarray(inp["k_b"], f)
    v_w = np.asarray(inp["v_w"], f); v_b = np.asarray(inp["v_b"], f)
    rho_w1 = np.asarray(inp["rho_w1"], f); rho_b1 = np.asarray(inp["rho_b1"], f)
    mean_w = np.asarray(inp["mean_w"], f); mean_b = np.asarray(inp["mean_b"], f)
    lstd_w = np.asarray(inp["lstd_w"], f); lstd_b = np.asarray(inp["lstd_b"], f)

    w1row = np.ascontiguousarray(np.tile(phi_w1, (4, 1)))            # [128,256]
    w2 = np.ascontiguousarray(np.concatenate([phi_w2[0:128], phi_w2[128:256]], axis=1))
    rv_full = v_w @ rho_w1                                            # [256,256]
    rv = np.ascontiguousarray(np.concatenate([rv_full[0:128], rv_full[128:256]], axis=1))
    rvb_full = 5.0 * (v_b @ rho_w1) + rho_b1                          # [256]
    mw_full = np.concatenate([mean_w, lstd_w], axis=1)                # [256,8]
    mw = np.ascontiguousarray(np.concatenate([mw_full[0:128], mw_full[128:256]], axis=1))
    qkw_full = np.concatenate([q_w, k_w], axis=1)                     # [256,2]
    qkw = np.ascontiguousarray(np.concatenate([qkw_full[0:128], qkw_full[128:256]], axis=1))
    selqk = np.zeros((2, 250), f)
    for n in range(5):
        for j in range(25):
            if j // 5 == n:
                selqk[0, 25 * n + j] = 1.0      # Q25[j] = q_n
            if j % 5 == n:
                selqk[1, 125 + 25 * n + j] = 1.0  # K25[j] = k_n
    qkb2 = np.array([[q_b[0]], [k_b[0]]], f)
    sumd = np.zeros((25, 5), f); seld = np.zeros((5, 25), f); sumw = np.zeros((25, 5), f)
    for n in range(5):
        for m in range(5):
            sumd[5 * n + m, n] = 1.0
            seld[n, 5 * n + m] = 1.0
            sumw[5 * n + m, m] = 1.0
    b1 = np.ascontiguousarray(phi_b1.reshape(2, 128).T)
    b2 = np.ascontiguousarray(phi_b2.reshape(2, 128).T)
    rvb = np.ascontiguousarray(rvb_full.reshape(2, 128).T)
    mb = np.ascontiguousarray(
        np.concatenate([mean_b, lstd_b]).reshape(8, 1))
    em5 = np.zeros((5, 640), f)
    for m in range(5):
        em5[m, 128 * m:128 * (m + 1)] = 1.0
    return dict(w1row=w1row, w2=w2, rv=rv, mw=mw, qkw=qkw, selqk=selqk,
                qkb2=qkb2, sumd=sumd, seld=seld, sumw=sumw, b1=b1, b2=b2,
                rvb=rvb, mb=mb, em5=em5)


def _run(obs, gp, agp, weights, n_mt, ncores, trace=False, repeat=1):
    global LAST_EXEC_NS
    key = (n_mt, repeat)
    if key not in _CACHE:
        _CACHE[key] = _build(n_mt, repeat)
    nc = _CACHE[key]
    rpc = n_mt * RMT
    in_maps = []
    for c in range(ncores):
        m = dict(weights)
        m["obs"] = np.ascontiguousarray(obs[c * rpc:(c + 1) * rpc])
        m["gp"] = np.ascontiguousarray(gp[c * rpc:(c + 1) * rpc])
        m["agp"] = np.ascontiguousarray(agp[c * rpc:(c + 1) * rpc])
        in_maps.append(m)
    res = run_bass_kernel_spmd(nc, in_maps, list(range(ncores)))
    LAST_EXEC_NS = res.exec_time_ns
    out = np.concatenate([res.results[c]["out"] for c in range(ncores)], axis=0)
    return out


def kernel(**inputs):
    f = np.float32
    obs = np.asarray(inputs["obs"], f)
    ag = np.asarray(inputs["ag"], f)
    g = np.asarray(inputs["g"], f)
    sem_ids = np.asarray(inputs["sem_ids"]).astype(np.int64)
    perm = sem_ids.reshape(-1)
    gp = np.ascontiguousarray(g[:, perm])
    agp = np.ascontiguousarray(ag[:, perm])
    weights = _prep_weights(inputs)
    n_mt = obs.shape[0] // (NCORES * RMT)
    out = _run(obs, gp, agp, weights, n_mt, NCORES)
    mean = np.ascontiguousarray(out[:, 0:4])
    logstd = np.ascontiguousarray(out[:, 4:8])
    return mean, logstd

